# revision 27
# baseline (speedup 1.0000x reference)
"""NT-Xent contrastive loss on 8 Trainium2 NeuronCores (v4: symmetric halving).

Reference: zz [4096, 2, 128] fp32 -> scalar fp32 loss.
  z = cat(zz[:,0], zz[:,1])           [8192, 128]
  zn = z / max(||z||, eps)
  sim = (zn @ zn.T) / 0.07
  loss = mean_i( log(sum_{j != i} exp(sim_ij)) - sim_{i, i±4096} )
(The positive-pair mask term cancels against the prepended pos logit, so
 only the self-diagonal needs excluding.)

v4 idea: sim is symmetric, so exp (the ACT-engine roofline) is only
needed on ~half the entries. Block the 8192x8192 sim into 64x64 tiles of
128x128. Core c (via input rotation) owns row tiles k=0..7 and computes
tiles (k, k+d) for d=0..32 only:
  - row sums of each exp'd tile: free via ACT accum_out,
  - column sums (= row sums of the transposed tile, owned by other
    rows/cores): a [128,1] E-stationary ones-matmul per tile on the
    otherwise idle PE engine, written into tail columns of the exp
    group's own (already consumed) PSUM tile, then DVE-added into an
    SBUF accumulator.
d=0 (self tile) and d=32 (tiles whose transpose is another core's d=32
tile) contribute row sums only. Each global row r then receives its
full sum as: own-strip row sums (d=0..32, cols r..r+32) + column-sum
contributions from tiles (r+e, r), e=33..63, computed by other cores.
The per-core partial vectors (row sums [128,8], col sums [128,38]) are
DMA'd out (36 KB) and combined on host: S_neg = total - selfexp, then
loss = mean(log(S_neg) - pos/T). Host also pre-normalizes (fp64 norms,
bf16 rounding) and pre-transposes z per core, so the device does no
transposes at all.

ACT does 264 tiles * 16K = 4.33M exps/core (~35us busy; measured
2176ns per [128,2048] exp+accum on HW) instead of 8.4M (~70us);
everything else (PE matmuls+colsums ~30us, DVE ~6us, 1.25MB DMA-in)
hides under it. The bench loop is unrolled 4 bodies per For_i
iteration over 4 buffer sets with input-DMA software rotation, so
DMA/prologue/tail overlap across bodies and the back-edge drain
amortizes; the C block leads each body (its chunks 0+4 head the DMA
queues) so the ACT stream starts immediately. Steady state measures
~37us/iteration vs the 87us baseline.
"""

import sys
import numpy as np

sys.path.insert(0, "/opt/trn_rl_repo")

B = 4096
N = 8192  # 2B
D = 128
NCORES = 8
TEMP = 0.07
SCALE = 1.0 / TEMP

NK = 8          # row tiles per core
NDA = 16        # group A: d = 0..15
NDB = 16        # group B: d = 16..31
GW = 2048       # group width (NDA*128)
TCOLS = 38      # colacc valid local col tiles: t = k+d, k<=7, 1<=d<=31
NT = 40         # znT col tiles actually used (strips span k..k+32, k<=7)
ZW = NT * 128   # 5120

LAST_RESULTS = None

# Bench-only ablation switches (break correctness; see ablate.py).
ABLATE_NO_COLSUM = False
ABLATE_NO_ACCUM = False
UNROLL = 4


def _build_bass(iters: int = 1):
    import concourse.tile as tile
    from concourse import mybir
    from concourse.bacc import Bacc
    from contextlib import ExitStack

    f32 = mybir.dt.float32
    bf16 = mybir.dt.bfloat16

    # Bacc (not raw Bass): its finalize() runs move_matmul_waits_to_ldweights
    # + generate_event_semaphores, which legalize multi-semaphore waits down
    # to the 1-wait-per-instruction TRN2 limit, and codegen for ISA-subclass
    # instructions. Raw Bass skips all of that and neuronxcc rejects the IR.
    nc = Bacc("TRN2", target_bir_lowering=False, debug=False,
              num_devices=NCORES)

    # Each core receives znT ROTATED (cols = local rows; local row x is
    # global row (c*1024+x) mod 8192) and pre-TRANSPOSED on host, so its
    # strips are always tiles (k, k+d), k=0..7, d=0..32, no wraparound.
    znT_in = nc.dram_tensor("znT", [D, ZW], bf16, kind="ExternalInput").ap()
    srow_out = nc.dram_tensor("srow_out", [128, NK], f32,
                              kind="ExternalOutput").ap()
    colacc_out = nc.dram_tensor("colacc_out", [128, TCOLS], f32,
                                kind="ExternalOutput").ap()

    with tile.TileContext(nc) as tc, ExitStack() as ctx:
        singles = ctx.enter_context(tc.tile_pool(name="singles", bufs=1))

        onesb = singles.tile([128, 1], bf16)
        warm = singles.tile([128, 1], f32)

        # One shared PSUM pool (2 x [128,2048]f32 = all 8 banks) and E pool
        # across loop bodies: slots keep alternating across the body
        # boundary, so the next body's first matmuls reuse banks freed two
        # groups earlier instead of serializing on the previous body's
        # final colsum chain.
        mpsum = ctx.enter_context(tc.tile_pool(name="mpsum", bufs=2,
                                               space="PSUM"))
        epool = ctx.enter_context(tc.tile_pool(name="epool", bufs=3))

        class BufSet:
            """Per-iteration-written tiles; two sets alternate across the
            bench loop so iteration i+1's DMA/prologue overlaps i's tail
            (a hardware For_i has static addresses, so buffer rotation must
            be unrolled by hand)."""

            def __init__(self, i):
                self.znT = singles.tile([D, ZW], bf16, name=f"znT{i}")
                self.colaccS = singles.tile([128, 64], f32, name=f"cac{i}")
                self.Sg = singles.tile([128, 2 * NK], f32, name=f"Sg{i}")
                self.SgC = singles.tile([128, NK], f32, name=f"SgC{i}")
                self.selfG = singles.tile([128, NK], f32, name=f"sG{i}")
                self.selfexp = singles.tile([128, NK], f32, name=f"sE{i}")
                self.srow = singles.tile([128, NK], f32, name=f"sr{i}")
                self.EC = singles.tile([128, NK * 128], bf16, name=f"EC{i}")

        def body_dma(S):
            # ---- stage input: 5 chunks on 2 queues. The C block runs
            # first in each body and needs chunks 0 and 4, so those head
            # the two queues; A0 needs chunk 1 next. Issued right after
            # the previous compute on S finishes reading znT, so the
            # transfer overlaps the other sets' compute (including across
            # the For_i back-edge, which cannot prefetch).
            for q, j in ((nc.sync, 0), (nc.gpsimd, 4), (nc.sync, 1),
                         (nc.gpsimd, 2), (nc.sync, 3)):
                q.dma_start(
                    out=S.znT[:, j * 1024:(j + 1) * 1024],
                    in_=znT_in[:, j * 1024:(j + 1) * 1024])

        def body(S):
            znT, colaccS, Sg, SgC = S.znT, S.colaccS, S.Sg, S.SgC
            selfG, selfexp, srow, EC = S.selfG, S.selfexp, S.srow, S.EC
            nc.vector.memset(colaccS[:], 0.0)

            if True:
                # Software-pipelined: colsum matmuls for group g are
                # emitted after the matmuls of group g+1, so the in-order
                # PE queue isn't blocked waiting on ACT(g).
                pending = []  # (ps, E, k, d0, nd)

                def flush_pending():
                    if not pending:
                        return
                    ps, E, k, d0, nd = pending.pop()
                    if ABLATE_NO_COLSUM:
                        return
                    # column sums for d in [max(d0,1), d0+nd): tile d's
                    # exp block E[:, (d-d0)*128:...] as stationary, ones
                    # moving -> out [128,1] = per-column sums, landed in
                    # tail cols of the consumed psum tile (bank 3).
                    dlo = max(d0, 1)
                    ncol = d0 + nd - dlo
                    base = GW - ncol
                    for i, d in enumerate(range(dlo, d0 + nd)):
                        j = d - d0
                        nc.tensor.matmul(
                            ps[:, base + i:base + i + 1],
                            E[:, j * 128:(j + 1) * 128], onesb[:])
                    t0 = k + dlo
                    nc.vector.tensor_add(
                        colaccS[:, t0:t0 + ncol], colaccS[:, t0:t0 + ncol],
                        ps[:, base:base + ncol])

                def group(k, d0):
                    # cols k*128+d0*128 .. +2047 (tiles k+d0 .. k+d0+15)
                    c0 = (k + d0) * 128
                    ps = mpsum.tile([128, GW], f32, name="ps")
                    for q in range(4):
                        nc.tensor.matmul(
                            ps[:, q * 512:(q + 1) * 512],
                            znT[:, k * 128:(k + 1) * 128],
                            znT[:, c0 + q * 512:c0 + (q + 1) * 512])
                    flush_pending()
                    E = epool.tile([128, GW], bf16, name="E")
                    acol = k if d0 == 0 else NK + k
                    nc.scalar.activation(
                        E[:], ps[:], mybir.ActivationFunctionType.Exp,
                        scale=SCALE,
                        accum_out=None if ABLATE_NO_ACCUM
                        else Sg[:, acol:acol + 1])
                    if d0 == 0:
                        # diagonal (~1.0 pre-scale) is the strict row max
                        # of the self Gram tile -> extract it exactly; its
                        # exp cancels bit-exactly out of the accum later.
                        nc.vector.tensor_reduce(
                            out=selfG[:, k:k + 1], in_=ps[:, 0:128],
                            axis=mybir.AxisListType.X,
                            op=mybir.AluOpType.max)
                    pending.append((ps, E, k, d0, NDA))

                # ---- C block first: d=32 tiles (k, k+32), row sums only.
                # Needs only znT chunks 0 and 4 (which head the DMA
                # queues), so each body's ACT stream starts on C while the
                # remaining chunks land; its DVE reduce overlaps the A
                # phase, and the A->B transition needs no special block.
                psC = mpsum.tile([128, GW], f32, name="ps")
                for k in range(NK):
                    nc.tensor.matmul(
                        psC[:, k * 128:(k + 1) * 128],
                        znT[:, k * 128:(k + 1) * 128],
                        znT[:, (k + 32) * 128:(k + 33) * 128])
                nc.scalar.activation(EC[:], psC[:, 0:NK * 128],
                                     mybir.ActivationFunctionType.Exp,
                                     scale=SCALE)
                EC3 = EC.rearrange("p (g x) -> p g x", x=128)
                nc.vector.tensor_reduce(out=SgC[:], in_=EC3[:],
                                        axis=mybir.AxisListType.X,
                                        op=mybir.AluOpType.add)

                for k in range(NK):
                    group(k, 0)
                # selfexp: selfG complete after the A phase; the Exp is
                # emitted here so it rides the ACT queue between A7 and B0.
                nc.scalar.activation(selfexp[:], selfG[:],
                                     mybir.ActivationFunctionType.Exp,
                                     scale=SCALE)
                for k in range(NK):
                    group(k, NDB)
                flush_pending()

            # ---- combine: srow = SgA + SgB + SgC - selfexp ----
            Sg3 = Sg.rearrange("p (h k) -> p h k", h=2)
            nc.vector.tensor_add(srow[:], Sg3[:, 0, :], Sg3[:, 1, :])
            nc.vector.tensor_add(srow[:], srow[:], SgC[:])
            nc.vector.tensor_sub(srow[:], srow[:], selfexp[:])

            nc.sync.dma_start(out=srow_out[:, :], in_=srow[:])
            nc.gpsimd.dma_start(out=colacc_out[:, :],
                                in_=colaccS[:, 1:1 + TCOLS])

        # Warm the Exp table once, overlapping the first input DMA, so no
        # activation ever pays the 1.3us ACT_TABLE_LOAD (Exp is the only
        # table function used, so it stays loaded across iterations).
        nc.vector.memset(onesb[:], 1.0)
        nc.scalar.activation(warm[:], onesb[:],
                             mybir.ActivationFunctionType.Exp)

        U = UNROLL
        if iters == 1:
            s0 = BufSet(0)
            body_dma(s0)
            body(s0)
        else:
            assert (iters - 1) % U == 0, f"expect iters = 1 + {U}k"
            sets = [BufSet(i) for i in range(U)]
            for s in sets:
                body_dma(s)
            body(sets[0])
            with tc.For_i(0, (iters - 1) // U, 1):
                for i in list(range(1, U)) + [0]:
                    body(sets[i])
                    body_dma(sets[i])

    # Bacc defers register allocation to compile(), which runs in
    # finalize(); run_bass_via_pjrt serializes the module as-is, so
    # without this neuronxcc sees reg_id=-1 ("Reg has not been allocated").
    nc.finalize()
    return nc


def _host_prep(zz: np.ndarray) -> np.ndarray:
    """Concat views and normalize rows (fp64 norms), round to bf16."""
    import ml_dtypes

    zz = np.asarray(zz, dtype=np.float32)
    z = np.concatenate([zz[:, 0, :], zz[:, 1, :]], axis=0)
    n = np.maximum(np.linalg.norm(z.astype(np.float64), axis=1,
                                  keepdims=True), 1e-8)
    zn = (z.astype(np.float64) / n).astype(np.float32)
    return zn.astype(ml_dtypes.bfloat16)


def _make_in_maps(znb: np.ndarray) -> list:
    return [{
        "znT": np.ascontiguousarray(np.roll(znb, -c * 1024, axis=0)[:ZW].T),
    } for c in range(NCORES)]


def _host_combine(znb: np.ndarray, results: list) -> np.ndarray:
    """Assemble S_neg[r] from per-core partials; loss = mean(ln(S)-pos/T)."""
    znf = znb.astype(np.float32)
    pos = np.einsum("rd,rd->r", znf, np.roll(znf, -B, axis=0))

    S = np.zeros(N, dtype=np.float64)
    p = np.arange(128)
    t = np.arange(1, 1 + TCOLS)
    for c in range(NCORES):
        srow = results[c]["srow_out"].astype(np.float64)     # [128, 8]
        S[c * 1024:(c + 1) * 1024] += srow.T.ravel()
        colacc = results[c]["colacc_out"].astype(np.float64)  # [128, 38]
        idx = (c * 1024 + t[:, None] * 128 + p[None, :]) % N
        np.add.at(S, idx, colacc.T)

    loss = np.mean(np.log(S) - pos.astype(np.float64) * SCALE)
    return np.array(loss, dtype=np.float32)


def kernel(zz: np.ndarray) -> np.ndarray:
    global LAST_RESULTS
    from concourse import bass_utils

    znb = _host_prep(zz)
    nc = _build_bass()
    res = bass_utils.run_bass_kernel_spmd(
        nc, _make_in_maps(znb), list(range(NCORES)), trace=False)
    LAST_RESULTS = res
    return _host_combine(znb, res.results)
